# revision 5
# baseline (speedup 1.0000x reference)
"""Trainium2 Bass kernel for the LiquidNeuralCell problem.

reference semantics:
    xw = einsum('btd,du->btu', x, W) + bias          # input projection
    h_t = tanh(xw_t + h_{t-1} @ R)                   # sequential scan over T
    out = stack of h_t                               # [B, T, U]

Strategy (8 NeuronCores, data-parallel over batch):
  - Each core gets B/8 = 8 batch rows; weights replicated.
  - Phase 1 (projection): x tiles loaded naturally, transposed on the PE
    (128x128 blocks), then x-stationary matmuls stream W. Bias folded in
    via a rank-1 (K=1) ones x bias matmul into the same PSUM group.
  - Phase 2 (scan): h kept transposed (hT, [u on partitions, batch free]).
    Per step the recurrent matmul is 4-way column-packed on the PE array:
    chunk j computes h_new[:, 256j:256j+256] in PE column strip j
    (tile_position (0, 32j)), with the xw_t contribution injected first
    via a K=8 identity matmul (start=True) so no separate DVE add is
    needed. tanh on ScalarE, then 4-way row-packed PE transposes rebuild
    hT for the next step.
"""

import sys
from contextlib import ExitStack

import numpy as np

for _p in ("/opt/trn_rl_repo",):
    if _p not in sys.path:
        sys.path.insert(0, _p)

import concourse.bass as bass
import concourse.tile as tile
from concourse import bacc, mybir
from concourse.bass_utils import run_bass_kernel_spmd
from concourse.masks import make_identity

F32 = mybir.dt.float32
AF = mybir.ActivationFunctionType

B, T, D, U = 64, 512, 2048, 1024
NCORES = 8
BS = B // NCORES          # batch rows per core (8)
KD = D // 128             # 16 projection k-tiles
KU = U // 128             # 8 recurrence k-tiles
NCH = 4                   # column-packed chunks per step
CW = U // NCH             # 256 chunk width


def build(nT: int = T) -> bass.Bass:
    nc = bacc.Bacc(None, target_bir_lowering=False)
    x = nc.declare_dram_parameter("x", [BS, nT, D], F32, isOutput=False)
    h0 = nc.declare_dram_parameter("h0", [BS, U], F32, isOutput=False)
    w = nc.declare_dram_parameter("w", [D, U], F32, isOutput=False)
    r = nc.declare_dram_parameter("r", [U, U], F32, isOutput=False)
    bias = nc.declare_dram_parameter("bias", [1, U], F32, isOutput=False)
    ys = nc.declare_dram_parameter("ys", [BS, nT, U], F32, isOutput=True)

    with tile.TileContext(nc) as tc, ExitStack() as ctx:
        dram = ctx.enter_context(tc.tile_pool(name="dram", bufs=1, space="DRAM"))
        xw_dram = dram.tile([BS, nT, U], F32)

        singles = ctx.enter_context(tc.tile_pool(name="singles", bufs=1))
        ident = singles.tile([128, 128], F32)
        make_identity(nc, ident)
        ones_row = singles.tile([1, 128], F32)
        nc.vector.memset(ones_row, 1.0)
        bias_sb = singles.tile([1, U], F32)
        nc.sync.dma_start(out=bias_sb, in_=bias[:, :])

        # ---------------- Phase 1: xw = x @ W + bias ----------------
        x_flat = x[:, :, :].flatten_outer_dims()        # [BS*nT, D]
        xw_flat = xw_dram[:, :, :].flatten_outer_dims() # [BS*nT, U]
        n_m = (BS * nT) // 128
        with (
            tc.tile_pool(name="wpool", bufs=1) as wpool,
            tc.tile_pool(name="p1", bufs=3) as p1,
            tc.tile_pool(name="p1o", bufs=3) as p1o,
            tc.tile_pool(name="p1psT", bufs=4, space="PSUM") as psT,
            tc.tile_pool(name="p1psM", bufs=2, space="PSUM") as psM,
        ):
            w_sb = wpool.tile([128, KD, U], F32)
            for k in range(KD):
                nc.sync.dma_start(out=w_sb[:, k, :], in_=w[k * 128:(k + 1) * 128, :])

            for m in range(n_m):
                x_tile = p1.tile([128, D], F32, tag="xtile")
                nc.sync.dma_start(out=x_tile, in_=x_flat[m * 128:(m + 1) * 128, :])
                xT = p1.tile([128, KD, 128], F32, tag="xT")
                for k in range(KD):
                    pt = psT.tile([128, 128], F32, tag="pt")
                    nc.tensor.transpose(pt, x_tile[:, k * 128:(k + 1) * 128], ident)
                    nc.vector.tensor_copy(xT[:, k, :], pt)
                for nh in range(2):
                    pm = psM.tile([128, 512], F32, tag="pm")
                    # bias: rank-1 ones^T x bias starts the accumulation group
                    nc.tensor.matmul(pm, ones_row[:, 0:128],
                                     bias_sb[:, nh * 512:(nh + 1) * 512],
                                     start=True, stop=False)
                    for k in range(KD):
                        nc.tensor.matmul(pm, xT[:, k, :],
                                         w_sb[:, k, nh * 512:(nh + 1) * 512],
                                         start=False, stop=(k == KD - 1))
                    o_sb = p1o.tile([128, 512], F32, tag="osb")
                    nc.vector.tensor_copy(o_sb, pm)
                    nc.sync.dma_start(
                        out=xw_flat[m * 128:(m + 1) * 128, nh * 512:(nh + 1) * 512],
                        in_=o_sb)

        # ---------------- Phase 2: the scan ----------------
        with (
            tc.tile_pool(name="rpool", bufs=1) as rpool,
            tc.tile_pool(name="s2", bufs=1) as s2,
            tc.tile_pool(name="hTp", bufs=2) as hTp,
            tc.tile_pool(name="hnp", bufs=3) as hnp,
            tc.tile_pool(name="xwp", bufs=4) as xwp,
            tc.tile_pool(name="psmm", bufs=2, space="PSUM") as psmm,
            tc.tile_pool(name="psT2", bufs=2, space="PSUM") as psT2,
        ):
            r_sb = rpool.tile([128, KU, U], F32)
            for k in range(KU):
                nc.sync.dma_start(out=r_sb[:, k, :], in_=r[k * 128:(k + 1) * 128, :])

            # initial hT from h0
            h0_sb = s2.tile([BS, U], F32)
            nc.sync.dma_start(out=h0_sb, in_=h0[:, :])
            pT0 = psT2.tile([128, KU * BS], F32, tag="pT")
            for k in range(KU):
                nc.tensor.transpose(pT0[:, k * BS:(k + 1) * BS],
                                    h0_sb[0:BS, k * 128:(k + 1) * 128],
                                    ident[0:BS, 0:BS],
                                    tile_position=(0, 0))
            hT = hTp.tile([128, KU * BS], F32, tag="hT")
            nc.vector.tensor_copy(hT, pT0)

            for t in range(nT):
                xw_sb = xwp.tile([BS, U], F32, tag="xw")
                nc.sync.dma_start(out=xw_sb, in_=xw_dram[:, t, :])

                pm = psmm.tile([128, CW], F32, tag="mm")
                for j in range(NCH):
                    out_sl = pm[32 * j:32 * j + BS, :]
                    # inject xw_t chunk via K=8 identity matmul (group start)
                    nc.tensor.matmul(out_sl, ident[0:BS, 0:BS],
                                     xw_sb[0:BS, j * CW:(j + 1) * CW],
                                     start=True, stop=False,
                                     tile_position=(0, 32 * j))
                    for k in range(KU):
                        nc.tensor.matmul(out_sl, hT[:, k * BS:(k + 1) * BS],
                                         r_sb[:, k, j * CW:(j + 1) * CW],
                                         start=False, stop=(k == KU - 1),
                                         tile_position=(0, 32 * j))

                hn = hnp.tile([128, CW], F32, tag="hn")
                for j in range(NCH):
                    nc.scalar.activation(hn[32 * j:32 * j + BS, :],
                                         pm[32 * j:32 * j + BS, :], AF.Tanh)

                pT = psT2.tile([128, KU * BS], F32, tag="pT")
                for j in range(NCH):
                    for half in range(CW // 128):
                        blk = (CW // 128) * j + half
                        nc.tensor.transpose(
                            pT[:, blk * BS:(blk + 1) * BS],
                            hn[32 * j:32 * j + BS, half * 128:(half + 1) * 128],
                            ident[32 * j:32 * j + BS, 32 * j:32 * j + BS],
                            tile_position=(32 * j, 0))
                hT = hTp.tile([128, KU * BS], F32, tag="hT")
                nc.vector.tensor_copy(hT, pT)

                for j in range(NCH):
                    nc.sync.dma_start(out=ys[:, t, j * CW:(j + 1) * CW],
                                      in_=hn[32 * j:32 * j + BS, :])

    nc.compile()
    return nc


_CACHE: dict = {}


def _get_prog() -> bass.Bass:
    if "nc" not in _CACHE:
        _CACHE["nc"] = build(T)
    return _CACHE["nc"]


def _in_maps(x, h0, kernel_w, recurrent_kernel, bias):
    maps = []
    for c in range(NCORES):
        sl = slice(c * BS, (c + 1) * BS)
        maps.append({
            "x": np.ascontiguousarray(x[sl], dtype=np.float32),
            "h0": np.ascontiguousarray(h0[sl], dtype=np.float32),
            "w": np.ascontiguousarray(kernel_w, dtype=np.float32),
            "r": np.ascontiguousarray(recurrent_kernel, dtype=np.float32),
            "bias": np.ascontiguousarray(bias, dtype=np.float32).reshape(1, U),
        })
    return maps


def kernel(x, h0, kernel, recurrent_kernel, bias):
    nc = _get_prog()
    maps = _in_maps(x, h0, kernel, recurrent_kernel, bias)
    res = run_bass_kernel_spmd(nc, maps, list(range(NCORES)))
    return np.concatenate([res.results[c]["ys"] for c in range(NCORES)], axis=0)
